# revision 25
# baseline (speedup 1.0000x reference)
"""Bidirectional attention kernel for Trainium2 (Bass/Tile), SPMD over 8 cores.

Per batch n (one batch per core):
    scores  = (lhs * w_lhs) @ (rhs * w_rhs).T          # [L, R]
            = (lhs * (w_lhs*w_rhs)) @ rhs.T            # diagonal scales compose
    E       = exp(scores)                              # no max-subtraction:
                                                       # |scores| < ~0.5 here
    lhs_ctx = (E @ rhs)   / rowsum(E)                  # row softmax folded into scale
    rhs_ctx = (E.T @ lhs) / colsum(E)                  # col softmax folded into scale
    out_lhs = [lhs | lhs_ctx],  out_rhs = [rhs | rhs_ctx]

Dual-S design: S^T is computed directly on the tensor engine from the same
resident fp8 operands instead of XBAR-transposing exp(S) (+33% PE work,
which has headroom; kills the serial Exp->transpose->cast chain and 8 MB of
SBUF<->SBUF DMA). Exp writes fp8 E and E^T directly; its accum_out yields
the row/col sums for free. All matmuls are fp8 DoubleRow (contraction
256/instr, FD=512): 1024 MMs/core, ~140us PE floor.

fp8 range: T1 carries 16*(w_lhs*w_rhs) to lift lhs*w2 (~1e-2) out of the
fp8e4 subnormal zone; the Exp activation applies scale=1/16 to compensate.

Asymmetric streaming schedule (per-engine queues execute IN ORDER, so the
emission order is the schedule):
1. lhs streams first (bf16 cast-DMA loads -> transpose -> fp8 casts),
   ~30us with PE idle -- the unavoidable head.
2. rhs streams; as chunk k lands, the FULL S^T row k runs (its moving
   operand, all of T1, is ready) -- PE works through the entire rhs
   stream. First half of rhs loads as f32 so the raw out_rhs half can be
   written from SBUF inside this window's DMA slack.
3. Natural-S row i + lhs_ctx row i (C1) interleave: C1 needs complete E^T
   (step 2) + rowsum(i) which natural row i just produced.
4. rhs_ctx rows (C2) close; DRAM->DRAM raw-half concats (lhs + second
   half of rhs) spread through steps 3-4 on the then-idle SWDGE queue.

E layouts are pair-blocked [tok%128, chunk, pair, 2, tok%128] so ctx
stationaries are contiguous 256B DoubleRow pair-blocks."""

import os
import sys

import numpy as np

for _p in ("/root/.axon_site/_ro/trn_rl_repo", "/opt/trn_rl_repo"):
    if os.path.isdir(_p) and _p not in sys.path:
        sys.path.append(_p)

N_CORES = 8
L, R, D = 2048, 2048, 1024


def build_program(L, R, D, repeat=1, phases="all"):
    from contextlib import ExitStack

    import concourse.bass as bass
    import concourse.mybir as mybir
    import concourse.tile as tile
    from concourse import bacc

    f32 = mybir.dt.float32
    bf16 = mybir.dt.bfloat16
    f8 = mybir.dt.float8e4
    DR = mybir.MatmulPerfMode.DoubleRow
    P = 128
    LC, RC, DC = L // P, R // P, D // P
    assert DC % 2 == 0 and RC % 2 == 0 and LC % 2 == 0
    QW = min(512, R)        # moving-operand width per matmul (scores FD)
    EW = min(1024, R)       # Exp width (psum tile, 2 banks)
    NH = R // EW            # Exp tiles per chunk-row
    EC = EW // P            # chunks per Exp tile
    QH = EW // QW           # QW-tiles per Exp tile
    MN = min(512, D)        # ctx matmul moving free width
    NMN = D // MN
    RSB = 0                 # all raw halves go DRAM->DRAM (keeps the rin f32
                            # path out of the stream; SBUF goes to staging)
    CH = min(8, LC)         # chunks per transpose group (amortizes the
                            # transpose-vs-DMA serialization window)

    nc = bacc.Bacc("TRN2", target_bir_lowering=False, debug=False)

    lhs = nc.dram_tensor("lhs", [L, D], f32, kind="ExternalInput")
    rhs = nc.dram_tensor("rhs", [R, D], f32, kind="ExternalInput")
    w_lhs = nc.dram_tensor("w_lhs", [1, D], f32, kind="ExternalInput")
    w_rhs = nc.dram_tensor("w_rhs", [1, D], f32, kind="ExternalInput")
    out_lhs = nc.dram_tensor("out_lhs", [L, 2 * D], f32, kind="ExternalOutput")
    out_rhs = nc.dram_tensor("out_rhs", [R, 2 * D], f32, kind="ExternalOutput")

    Exp = mybir.ActivationFunctionType.Exp
    Copy = mybir.ActivationFunctionType.Copy
    mult = mybir.AluOpType.mult
    add = mybir.AluOpType.add

    with tile.TileContext(nc) as tc, ExitStack() as ctx:
        const = ctx.enter_context(tc.tile_pool(name="const", bufs=1))
        res = ctx.enter_context(tc.tile_pool(name="res", bufs=1))
        tbp = ctx.enter_context(tc.tile_pool(name="tbp", bufs=2))
        ttp = ctx.enter_context(tc.tile_pool(name="ttp", bufs=2))
        outp = ctx.enter_context(tc.tile_pool(name="outp", bufs=3))
        scal = ctx.enter_context(tc.tile_pool(name="scal", bufs=4))

        # Resident fp8 transposed operands, [d%128, d//128, tok]: moving
        # slices merge to 3D [128, 2, QW]; stationary slices are [128, 2,
        # 128] with pair stride L.
        T1D = res.tile([P, DC, L], f8, tag="T1D")
        T2D = res.tile([P, DC, R], f8, tag="T2D")
        # E matrices, pair-blocked so ctx stationaries are contiguous:
        # Ef8[l%128, r_chunk, l_pair, l_parity, r%128] = exp(S)[l, r]
        Ef8 = res.tile([P, RC, LC // 2, 2, P], f8, tag="Ef8")
        ETf8 = res.tile([P, LC, RC // 2, 2, P], f8, tag="ETf8")
        lhsb8 = res.tile([P, LC, D], f8, tag="lhsb8")
        rhsb8 = res.tile([P, RC, D], f8, tag="rhsb8")
        rsum = res.tile([P, LC, NH], f32, tag="rsum")
        csum = res.tile([P, RC, NH], f32, tag="csum")

        psS = ctx.enter_context(tc.tile_pool(name="psS", bufs=2, space="PSUM"))
        psC = ctx.enter_context(tc.tile_pool(name="psC", bufs=2, space="PSUM"))

        for rep in range(repeat):
            # w2 = 16 * w_lhs * w_rhs in [d%128, d//128] layout
            wlT = const.tile([P, DC], f32, tag="wlT")
            wrT = const.tile([P, DC], f32, tag="wrT")
            w2T = const.tile([P, DC], f32, tag="w2T")
            # scatter-gather layout loads (1024 tiny descriptors each) go on
            # the ACT HWDGE ring, which is idle until the first Exp -- on the
            # Pool queue they would head-of-line block every input load.
            nc.scalar.dma_start(wlT[:], w_lhs[0, :].rearrange("(dc di) -> di dc", di=P))
            nc.scalar.dma_start(wrT[:], w_rhs[0, :].rearrange("(dc di) -> di dc", di=P))

            def emit_w2():
                # emitted AFTER the lhs stream: the gathers above finish
                # under it, so nothing ever waits on w2T at a queue head
                nc.vector.tensor_mul(w2T[:], wlT[:], wrT[:])
                nc.scalar.activation(w2T[:], w2T[:], Copy, scale=16.0)

            def prep_l_group(g):
                c0 = g * CH
                tb4 = tbp.tile([P, CH, D], bf16, tag="tb4", name=f"tb4_l{g}")
                for j in range(CH):
                    c = c0 + j
                    nc.gpsimd.dma_start(tb4[:, j, :], lhs[c * P:(c + 1) * P, :])
                tT4 = ttp.tile([P, CH, DC, P], bf16, tag="tT4", name=f"tT4_l{g}")
                nc.sync.dma_start_transpose(tT4[:], tb4[:])
                for j in range(CH):
                    c = c0 + j
                    nc.vector.tensor_copy(T1D[:, :, c * P:(c + 1) * P],
                                          tT4[:, j, :, :])
                    nc.vector.tensor_copy(lhsb8[:, c, :], tb4[:, j, :])

            def prep_r_group(g):
                c0 = g * CH
                tb4 = tbp.tile([P, CH, D], bf16, tag="tb4", name=f"tb4_r{g}")
                rins = {}
                for j in range(CH):
                    c = c0 + j
                    if c < RSB:
                        # f32 load; raw half written from SBUF (after the
                        # transpose so it never HOL-blocks the SP ring)
                        rin = inp.tile([P, D], f32, tag="rin", name=f"rin_r{c}")
                        nc.gpsimd.dma_start(rin[:], rhs[c * P:(c + 1) * P, :])
                        nc.vector.tensor_copy(tb4[:, j, :], rin[:])
                        rins[j] = rin
                    else:
                        nc.gpsimd.dma_start(tb4[:, j, :], rhs[c * P:(c + 1) * P, :])
                tT4 = ttp.tile([P, CH, DC, P], bf16, tag="tT4", name=f"tT4_r{g}")
                nc.sync.dma_start_transpose(tT4[:], tb4[:])
                for j, rin in rins.items():
                    c = c0 + j
                    nc.sync.dma_start(out_rhs[c * P:(c + 1) * P, 0:D], rin[:])
                w2b = w2T[:, :, None].to_broadcast((P, DC, P))
                for j in range(CH):
                    c = c0 + j
                    nc.vector.tensor_tensor(T2D[:, :, c * P:(c + 1) * P],
                                            tT4[:, j, :, :], w2b, mult)
                    nc.vector.tensor_copy(rhsb8[:, c, :], tb4[:, j, :])

            def score_half(stat_D, mov_D, out_e, out_sum, c, h):
                """Half-row (c, h) of scores + Exp into the blocked E layout."""
                ps = psS.tile([P, EW], f32, tag="psS", name=f"ps_{c}_{h}")
                for dcp in range(0, DC, 2):
                    for q in range(QH):
                        t = h * QH + q
                        nc.tensor.matmul(
                            ps[:, q * QW:(q + 1) * QW],
                            stat_D[:, dcp:dcp + 2, c * P:(c + 1) * P],
                            mov_D[:, dcp:dcp + 2, t * QW:(t + 1) * QW],
                            start=(dcp == 0), stop=(dcp == DC - 2),
                            perf_mode=DR,
                        )
                nc.scalar.activation(
                    out_e[:, h * EC:(h + 1) * EC, c // 2, c % 2, :],
                    ps[:], Exp, scale=0.0625,
                    accum_out=out_sum[:, c, h:h + 1],
                )

            def ctx_row(stat_e, mov_nat, sums, out, CN, c, pool, lbl):
                """ctx row c: (E-slice @ mov) / sum -> out[c-chunk, D:2D]."""
                pc = pool.tile([P, D], f32, tag=pool.name, name=f"pc_{lbl}_{c}")
                for kp in range(0, CN, 2):
                    for q in range(NMN):
                        nc.tensor.matmul(
                            pc[:, q * MN:(q + 1) * MN],
                            stat_e[:, c, kp // 2, :, :],
                            mov_nat[:, kp:kp + 2, q * MN:(q + 1) * MN],
                            start=(kp == 0), stop=(kp == CN - 2),
                            perf_mode=DR,
                        )
                tot = scal.tile([P, 1], f32, tag="tot", name=f"tot_{lbl}{c}")
                rec = scal.tile([P, 1], f32, tag="rec", name=f"rec_{lbl}{c}")
                nc.vector.tensor_reduce(tot[:], sums[:, c, :], mybir.AxisListType.X, add)
                nc.vector.reciprocal(rec[:], tot[:])
                co = outp.tile([P, D], f32, tag="ctxo", name=f"co_{lbl}{c}")
                nc.vector.tensor_scalar_mul(co[:], pc[:], rec[:])
                nc.sync.dma_start(out[c * P:(c + 1) * P, D:2 * D], co[:])

            # DRAM->DRAM raw-half concats (all lhs + second half of rhs),
            # spread one per back-phase row on the then-idle SWDGE queue.
            concats = [(out_lhs, lhs, c) for c in range(LC)] + [
                (out_rhs, rhs, c) for c in range(RSB, RC)
            ]

            def pop_concat():
                # on the SP ring: the ctx-write FIFO ahead of each concat
                # paces it into the back phases (on the dep-free Pool queue
                # they would all fire during the input streams)
                if concats:
                    out, src, c = concats.pop(0)
                    nc.sync.dma_start(
                        out[c * P:(c + 1) * P, 0:D], src[c * P:(c + 1) * P, :]
                    )

            # 1. lhs stream
            for g in range(LC // CH):
                prep_l_group(g)
            emit_w2()
            # 2. rhs stream + S^T rows
            for g in range(RC // CH):
                prep_r_group(g)
                for k in range(g * CH, (g + 1) * CH):
                    for h in range(NH):
                        score_half(T2D, T1D, ETf8, csum, k, h)
            # 3. natural-S rows + C1 rows
            for i in range(LC):
                for h in range(NH):
                    score_half(T1D, T2D, Ef8, rsum, i, h)
                if phases != "sonly":
                    ctx_row(ETf8, rhsb8, rsum, out_lhs, RC, i, psC, "c1")
                    pop_concat()
            if phases == "sonly":
                continue
            # 4. C2 rows
            for k in range(RC):
                ctx_row(Ef8, lhsb8, csum, out_rhs, LC, k,
                        psC if k % 2 else psS, "c2")
                pop_concat()
            while concats:
                pop_concat()

    nc.compile()
    return nc


_program = None


def _get_program():
    global _program
    if _program is None:
        _program = build_program(L, R, D)
    return _program


def kernel(lhs, rhs, w_lhs, w_rhs):
    from concourse.bass_utils import run_bass_kernel_spmd

    lhs = np.asarray(lhs, dtype=np.float32)
    rhs = np.asarray(rhs, dtype=np.float32)
    wl = np.asarray(w_lhs, dtype=np.float32).reshape(1, D)
    wr = np.asarray(w_rhs, dtype=np.float32).reshape(1, D)

    nc = _get_program()
    in_maps = [
        {"lhs": np.ascontiguousarray(lhs[c]), "rhs": np.ascontiguousarray(rhs[c]),
         "w_lhs": wl, "w_rhs": wr}
        for c in range(N_CORES)
    ]
    res = run_bass_kernel_spmd(nc, in_maps, core_ids=list(range(N_CORES)))
    out_lhs = np.stack([res.results[c]["out_lhs"] for c in range(N_CORES)])
    out_rhs = np.stack([res.results[c]["out_rhs"] for c in range(N_CORES)])
    return out_lhs, out_rhs


# revision 26
# speedup vs baseline: 210.5924x; 210.5924x over previous
"""Bidirectional attention kernel for Trainium2 (Bass/Tile), SPMD over 8 cores.

Per batch n (one batch per core):
    scores  = (lhs * w_lhs) @ (rhs * w_rhs).T          # [L, R]
            = (lhs * (w_lhs*w_rhs)) @ rhs.T            # diagonal scales compose
    E       = exp(scores)                              # no max-subtraction:
                                                       # |scores| < ~0.5 here
    lhs_ctx = (E @ rhs)   / rowsum(E)                  # row softmax folded into scale
    rhs_ctx = (E.T @ lhs) / colsum(E)                  # col softmax folded into scale
    out_lhs = [lhs | lhs_ctx],  out_rhs = [rhs | rhs_ctx]

Dual-S design: S^T is computed directly on the tensor engine from the same
resident fp8 operands instead of XBAR-transposing exp(S) (+33% PE work,
which has headroom; kills the serial Exp->transpose->cast chain and 8 MB of
SBUF<->SBUF DMA). Exp writes fp8 E and E^T directly; its accum_out yields
the row/col sums for free. All matmuls are fp8 DoubleRow (contraction
256/instr, FD=512): 1024 MMs/core, ~140us PE floor.

fp8 range: T1 carries 16*(w_lhs*w_rhs) to lift lhs*w2 (~1e-2) out of the
fp8e4 subnormal zone; the Exp activation applies scale=1/16 to compensate.

Asymmetric streaming schedule (per-engine queues execute IN ORDER, so the
emission order is the schedule):
1. lhs streams first (bf16 cast-DMA loads -> transpose -> fp8 casts),
   ~30us with PE idle -- the unavoidable head.
2. rhs streams; as chunk k lands, the FULL S^T row k runs (its moving
   operand, all of T1, is ready) -- PE works through the entire rhs
   stream. First half of rhs loads as f32 so the raw out_rhs half can be
   written from SBUF inside this window's DMA slack.
3. Natural-S row i + lhs_ctx row i (C1) interleave: C1 needs complete E^T
   (step 2) + rowsum(i) which natural row i just produced.
4. rhs_ctx rows (C2) close; DRAM->DRAM raw-half concats (lhs + second
   half of rhs) spread through steps 3-4 on the then-idle SWDGE queue.

E layouts are pair-blocked [tok%128, chunk, pair, 2, tok%128] so ctx
stationaries are contiguous 256B DoubleRow pair-blocks."""

import os
import sys

import numpy as np

for _p in ("/root/.axon_site/_ro/trn_rl_repo", "/opt/trn_rl_repo"):
    if os.path.isdir(_p) and _p not in sys.path:
        sys.path.append(_p)

N_CORES = 8
L, R, D = 2048, 2048, 1024


def build_program(L, R, D, repeat=1, phases="all"):
    from contextlib import ExitStack

    import concourse.bass as bass
    import concourse.mybir as mybir
    import concourse.tile as tile
    from concourse import bacc

    f32 = mybir.dt.float32
    bf16 = mybir.dt.bfloat16
    f8 = mybir.dt.float8e4
    DR = mybir.MatmulPerfMode.DoubleRow
    P = 128
    LC, RC, DC = L // P, R // P, D // P
    assert DC % 2 == 0 and RC % 2 == 0 and LC % 2 == 0
    QW = min(512, R)        # moving-operand width per matmul (scores FD)
    EW = min(1024, R)       # Exp width (psum tile, 2 banks)
    NH = R // EW            # Exp tiles per chunk-row
    EC = EW // P            # chunks per Exp tile
    QH = EW // QW           # QW-tiles per Exp tile
    MN = min(512, D)        # ctx matmul moving free width
    NMN = D // MN
    RSB = 0                 # all raw halves go DRAM->DRAM (keeps the rin f32
                            # path out of the stream; SBUF goes to staging)
    CH = min(8, LC)         # chunks per transpose group (amortizes the
                            # transpose-vs-DMA serialization window)

    nc = bacc.Bacc("TRN2", target_bir_lowering=False, debug=False)

    lhs = nc.dram_tensor("lhs", [L, D], f32, kind="ExternalInput")
    rhs = nc.dram_tensor("rhs", [R, D], f32, kind="ExternalInput")
    w_lhs = nc.dram_tensor("w_lhs", [1, D], f32, kind="ExternalInput")
    w_rhs = nc.dram_tensor("w_rhs", [1, D], f32, kind="ExternalInput")
    out_lhs = nc.dram_tensor("out_lhs", [L, 2 * D], f32, kind="ExternalOutput")
    out_rhs = nc.dram_tensor("out_rhs", [R, 2 * D], f32, kind="ExternalOutput")

    Exp = mybir.ActivationFunctionType.Exp
    Copy = mybir.ActivationFunctionType.Copy
    mult = mybir.AluOpType.mult
    add = mybir.AluOpType.add

    with tile.TileContext(nc) as tc, ExitStack() as ctx:
        const = ctx.enter_context(tc.tile_pool(name="const", bufs=1))
        res = ctx.enter_context(tc.tile_pool(name="res", bufs=1))
        tbp = ctx.enter_context(tc.tile_pool(name="tbp", bufs=2))
        ttp = ctx.enter_context(tc.tile_pool(name="ttp", bufs=2))
        outp = ctx.enter_context(tc.tile_pool(name="outp", bufs=3))
        scal = ctx.enter_context(tc.tile_pool(name="scal", bufs=4))

        # Resident fp8 transposed operands, [d%128, d//128, tok]: moving
        # slices merge to 3D [128, 2, QW]; stationary slices are [128, 2,
        # 128] with pair stride L.
        T1D = res.tile([P, DC, L], f8, tag="T1D")
        T2D = res.tile([P, DC, R], f8, tag="T2D")
        # E matrices, pair-blocked so ctx stationaries are contiguous:
        # Ef8[l%128, r_chunk, l_pair, l_parity, r%128] = exp(S)[l, r]
        Ef8 = res.tile([P, RC, LC // 2, 2, P], f8, tag="Ef8")
        ETf8 = res.tile([P, LC, RC // 2, 2, P], f8, tag="ETf8")
        lhsb8 = res.tile([P, LC, D], f8, tag="lhsb8")
        rhsb8 = res.tile([P, RC, D], f8, tag="rhsb8")
        rsum = res.tile([P, LC, NH], f32, tag="rsum")
        csum = res.tile([P, RC, NH], f32, tag="csum")

        psS = ctx.enter_context(tc.tile_pool(name="psS", bufs=2, space="PSUM"))
        psC = ctx.enter_context(tc.tile_pool(name="psC", bufs=2, space="PSUM"))

        for rep in range(repeat):
            # w2 = 16 * w_lhs * w_rhs in [d%128, d//128] layout
            wlT = const.tile([P, DC], f32, tag="wlT")
            wrT = const.tile([P, DC], f32, tag="wrT")
            w2T = const.tile([P, DC], f32, tag="w2T")
            # scatter-gather layout loads (1024 tiny descriptors each) go on
            # the ACT HWDGE ring, which is idle until the first Exp -- on the
            # Pool queue they would head-of-line block every input load.
            nc.scalar.dma_start(wlT[:], w_lhs[0, :].rearrange("(dc di) -> di dc", di=P))
            nc.scalar.dma_start(wrT[:], w_rhs[0, :].rearrange("(dc di) -> di dc", di=P))

            def emit_w2():
                # emitted AFTER the lhs stream: the gathers above finish
                # under it, so nothing ever waits on w2T at a queue head
                nc.vector.tensor_mul(w2T[:], wlT[:], wrT[:])
                nc.scalar.activation(w2T[:], w2T[:], Copy, scale=16.0)

            def prep_l_group(g):
                c0 = g * CH
                tb4 = tbp.tile([P, CH, D], bf16, tag="tb4", name=f"tb4_l{g}")
                for j in range(CH):
                    c = c0 + j
                    nc.gpsimd.dma_start(tb4[:, j, :], lhs[c * P:(c + 1) * P, :])
                tT4 = ttp.tile([P, CH, DC, P], bf16, tag="tT4", name=f"tT4_l{g}")
                for j in range(CH):
                    nc.sync.dma_start_transpose(tT4[:, j, :, :], tb4[:, j, :])
                for j in range(CH):
                    c = c0 + j
                    nc.vector.tensor_copy(T1D[:, :, c * P:(c + 1) * P],
                                          tT4[:, j, :, :])
                    nc.vector.tensor_copy(lhsb8[:, c, :], tb4[:, j, :])

            def prep_r_group(g):
                c0 = g * CH
                tb4 = tbp.tile([P, CH, D], bf16, tag="tb4", name=f"tb4_r{g}")
                rins = {}
                for j in range(CH):
                    c = c0 + j
                    if c < RSB:
                        # f32 load; raw half written from SBUF (after the
                        # transpose so it never HOL-blocks the SP ring)
                        rin = inp.tile([P, D], f32, tag="rin", name=f"rin_r{c}")
                        nc.gpsimd.dma_start(rin[:], rhs[c * P:(c + 1) * P, :])
                        nc.vector.tensor_copy(tb4[:, j, :], rin[:])
                        rins[j] = rin
                    else:
                        nc.gpsimd.dma_start(tb4[:, j, :], rhs[c * P:(c + 1) * P, :])
                tT4 = ttp.tile([P, CH, DC, P], bf16, tag="tT4", name=f"tT4_r{g}")
                for j in range(CH):
                    nc.sync.dma_start_transpose(tT4[:, j, :, :], tb4[:, j, :])
                for j, rin in rins.items():
                    c = c0 + j
                    nc.sync.dma_start(out_rhs[c * P:(c + 1) * P, 0:D], rin[:])
                w2b = w2T[:, :, None].to_broadcast((P, DC, P))
                for j in range(CH):
                    c = c0 + j
                    nc.vector.tensor_tensor(T2D[:, :, c * P:(c + 1) * P],
                                            tT4[:, j, :, :], w2b, mult)
                    nc.vector.tensor_copy(rhsb8[:, c, :], tb4[:, j, :])

            def score_half(stat_D, mov_D, out_e, out_sum, c, h):
                """Half-row (c, h) of scores + Exp into the blocked E layout."""
                ps = psS.tile([P, EW], f32, tag="psS", name=f"ps_{c}_{h}")
                for dcp in range(0, DC, 2):
                    for q in range(QH):
                        t = h * QH + q
                        nc.tensor.matmul(
                            ps[:, q * QW:(q + 1) * QW],
                            stat_D[:, dcp:dcp + 2, c * P:(c + 1) * P],
                            mov_D[:, dcp:dcp + 2, t * QW:(t + 1) * QW],
                            start=(dcp == 0), stop=(dcp == DC - 2),
                            perf_mode=DR,
                        )
                nc.scalar.activation(
                    out_e[:, h * EC:(h + 1) * EC, c // 2, c % 2, :],
                    ps[:], Exp, scale=0.0625,
                    accum_out=out_sum[:, c, h:h + 1],
                )

            def ctx_row(stat_e, mov_nat, sums, out, CN, c, pool, lbl):
                """ctx row c: (E-slice @ mov) / sum -> out[c-chunk, D:2D]."""
                pc = pool.tile([P, D], f32, tag=pool.name, name=f"pc_{lbl}_{c}")
                for kp in range(0, CN, 2):
                    for q in range(NMN):
                        nc.tensor.matmul(
                            pc[:, q * MN:(q + 1) * MN],
                            stat_e[:, c, kp // 2, :, :],
                            mov_nat[:, kp:kp + 2, q * MN:(q + 1) * MN],
                            start=(kp == 0), stop=(kp == CN - 2),
                            perf_mode=DR,
                        )
                tot = scal.tile([P, 1], f32, tag="tot", name=f"tot_{lbl}{c}")
                rec = scal.tile([P, 1], f32, tag="rec", name=f"rec_{lbl}{c}")
                nc.vector.tensor_reduce(tot[:], sums[:, c, :], mybir.AxisListType.X, add)
                nc.vector.reciprocal(rec[:], tot[:])
                co = outp.tile([P, D], f32, tag="ctxo", name=f"co_{lbl}{c}")
                nc.vector.tensor_scalar_mul(co[:], pc[:], rec[:])
                nc.sync.dma_start(out[c * P:(c + 1) * P, D:2 * D], co[:])

            # DRAM->DRAM raw-half concats (all lhs + second half of rhs),
            # spread one per back-phase row on the then-idle SWDGE queue.
            concats = [(out_lhs, lhs, c) for c in range(LC)] + [
                (out_rhs, rhs, c) for c in range(RSB, RC)
            ]

            def pop_concat():
                # on the SP ring: the ctx-write FIFO ahead of each concat
                # paces it into the back phases (on the dep-free Pool queue
                # they would all fire during the input streams)
                if concats:
                    out, src, c = concats.pop(0)
                    nc.sync.dma_start(
                        out[c * P:(c + 1) * P, 0:D], src[c * P:(c + 1) * P, :]
                    )

            # 1. lhs stream
            for g in range(LC // CH):
                prep_l_group(g)
            emit_w2()
            # 2. rhs stream + S^T rows
            for g in range(RC // CH):
                prep_r_group(g)
                for k in range(g * CH, (g + 1) * CH):
                    for h in range(NH):
                        score_half(T2D, T1D, ETf8, csum, k, h)
            # 3. natural-S rows + C1 rows
            for i in range(LC):
                for h in range(NH):
                    score_half(T1D, T2D, Ef8, rsum, i, h)
                if phases != "sonly":
                    ctx_row(ETf8, rhsb8, rsum, out_lhs, RC, i, psC, "c1")
                    pop_concat()
            if phases == "sonly":
                continue
            # 4. C2 rows
            for k in range(RC):
                ctx_row(Ef8, lhsb8, csum, out_rhs, LC, k,
                        psC if k % 2 else psS, "c2")
                pop_concat()
            while concats:
                pop_concat()

    nc.compile()
    return nc


_program = None


def _get_program():
    global _program
    if _program is None:
        _program = build_program(L, R, D)
    return _program


def kernel(lhs, rhs, w_lhs, w_rhs):
    from concourse.bass_utils import run_bass_kernel_spmd

    lhs = np.asarray(lhs, dtype=np.float32)
    rhs = np.asarray(rhs, dtype=np.float32)
    wl = np.asarray(w_lhs, dtype=np.float32).reshape(1, D)
    wr = np.asarray(w_rhs, dtype=np.float32).reshape(1, D)

    nc = _get_program()
    in_maps = [
        {"lhs": np.ascontiguousarray(lhs[c]), "rhs": np.ascontiguousarray(rhs[c]),
         "w_lhs": wl, "w_rhs": wr}
        for c in range(N_CORES)
    ]
    res = run_bass_kernel_spmd(nc, in_maps, core_ids=list(range(N_CORES)))
    out_lhs = np.stack([res.results[c]["out_lhs"] for c in range(N_CORES)])
    out_rhs = np.stack([res.results[c]["out_rhs"] for c in range(N_CORES)])
    return out_lhs, out_rhs
